# revision 2
# baseline (speedup 1.0000x reference)
"""Autoregressive GRU on 8 TRN2 NeuronCores — V1 pipeline rework.

Data-parallel: batch B=512 split 64 rows/core; T=128 sequential steps local
per core. Step 0 (zero input) is computed on the HOST (no v0 weight matrix,
one less device step); the device runs steps 1..127 with one fused weight.

Key algebra (Keras GRU, reset_after=True, gate order [z, r, h]):
  step t>=1: inp == h  ->  one matmul against host-prefused
  V = [Wr+Ur | Uh | Wh | Wz+Uz]  [D, 4D], bank order [r | hh | xh | z], then
  r = sigmoid(rb); hhat = tanh(xhb + r*hhb); z = sigmoid(zb)
  h_new = hhat + z*(h - hhat)

V1 changes vs the 1.08ms baseline (measured step 6490ns):
- q = xh + r*hh is accumulated BY THE PE: a 5th matmul ident.T @ p with
  start=False into the open xh PSUM bank. Drops a 688ns DVE op from the
  chain and starts tanh ~450ns earlier.
- z-bank is split into two 256-wide PSUM tiles (f32r still 1 cycle/row at
  N=256) so sigmoid(z0) starts a half-bank earlier.
- The recurrent state hT is 4 chunk tiles [128,64]; per chunk:
  tt_k = z*s chunk -> PE transpose -> hT_k = copy(hhat^T_k) + tt^T_k
  (copy on ScalarE, add on DVE), so the NEXT step's k-th matmul starts as
  soon as chunk k is assembled instead of after the full [128,256] state.
- Output DMA'd as bf16 straight from h_new (host casts to f32; h is already
  bf16 so values are identical) — frees the 721ns/step ScalarE f32 copy.
- Weights DMA'd per-bank so step 1's r-matmuls start after 1MB, not 4MB.
- PE filler transposes in the two tail idle windows keep the HAM activity
  monitor from re-throttling the PE clock (see baseline notes: idle windows
  re-gate the PE to 1.2 GHz; heavy regular-matmul filler tips the chip into
  P0 at 2.0 GHz, so filler must be transpose-mode reads of resident SBUF).
"""

import numpy as np
import ml_dtypes

B, D, T = 512, 512, 128
NCORES = 8
BLOC = B // NCORES  # 64
P = 128
KC = D // P  # 4 K-chunks
GW = 4 * D  # 2048 fused gate columns: [r | hh | xh | z]

_BF16 = ml_dtypes.bfloat16

TRACE = False
TMPDIR = None
LAST = {}


def _prepare_weights(W, U, b):
    """Host-side fusion. Returns (V, V0, bias) in math layout.

    V bank order [r | hh | xh | z]; V0 kept for numpy_sim compatibility."""
    Wz, Wr, Wh = W[:, :D], W[:, D : 2 * D], W[:, 2 * D :]
    Uz, Ur, Uh = U[:, :D], U[:, D : 2 * D], U[:, 2 * D :]
    V = np.concatenate([Wr + Ur, Uh, Wh, Wz + Uz], axis=1)  # [D, GW]
    V0 = np.concatenate([Ur, Uh, np.zeros_like(Wh), Uz], axis=1)
    b0, b1 = b[0], b[1]
    bias = np.concatenate(
        [b0[D : 2 * D] + b1[D : 2 * D], b1[2 * D :], b0[2 * D :], b0[:D] + b1[:D]]
    )  # [GW], order [r | hh | xh | z]
    return V, V0, bias


def _bank_layout(Vb):
    # Vb [D, w] -> dev [P, KC*w]: dev[p, k*w + j] = Vb[k*128 + p, j]
    w = Vb.shape[1]
    return np.ascontiguousarray(
        Vb.reshape(KC, P, w).transpose(1, 0, 2).reshape(P, KC * w)
    )


def _host_step0(x, W, U, b):
    """h1 = gru_cell(inp=0, h=x) in f32 on the host."""
    gx = np.broadcast_to(b[0], (x.shape[0], 3 * D))  # 0 @ W + b0
    gh = x @ U + b[1]
    xz, xr, xh = gx[:, :D], gx[:, D : 2 * D], gx[:, 2 * D :]
    hz, hr, hh = gh[:, :D], gh[:, D : 2 * D], gh[:, 2 * D :]
    z = 1.0 / (1.0 + np.exp(-(xz + hz)))
    r = 1.0 / (1.0 + np.exp(-(xr + hr)))
    hhat = np.tanh(xh + r * hh)
    return z * x + (1.0 - z) * hhat


_CACHE = {}


def _build(has_bias: bool):
    import concourse.mybir as mybir
    import concourse.tile as tile
    from concourse import bacc
    from concourse.masks import make_identity

    f32 = mybir.dt.float32
    f32r = mybir.dt.float32r
    bf16 = mybir.dt.bfloat16
    AF = mybir.ActivationFunctionType

    TDEV = T - 1  # 127 device steps

    nc = bacc.Bacc(
        "TRN2", target_bir_lowering=False, debug=False, num_devices=NCORES
    )
    vr_d = nc.dram_tensor("vr", [P, KC * 512], f32r, kind="ExternalInput").ap()
    vhh_d = nc.dram_tensor("vhh", [P, KC * 512], f32r, kind="ExternalInput").ap()
    vxh_d = nc.dram_tensor("vxh", [P, KC * 512], f32r, kind="ExternalInput").ap()
    vz0_d = nc.dram_tensor("vz0", [P, KC * 256], f32r, kind="ExternalInput").ap()
    vz1_d = nc.dram_tensor("vz1", [P, KC * 256], f32r, kind="ExternalInput").ap()
    h0_d = nc.dram_tensor("h0", [BLOC, D], bf16, kind="ExternalInput").ap()
    h0T_d = nc.dram_tensor("h0T", [P, KC * BLOC], f32r, kind="ExternalInput").ap()
    if has_bias:
        bias_d = nc.dram_tensor("bias", [BLOC, GW], f32, kind="ExternalInput").ap()
    out_d = nc.dram_tensor("out", [BLOC, TDEV, D], bf16, kind="ExternalOutput").ap()

    with tile.TileContext(nc) as tc:
        with (
            tc.tile_pool(name="const", bufs=1) as cpool,
            tc.tile_pool(name="state", bufs=3) as spool,
            tc.tile_pool(name="statet", bufs=2) as stpool,
            tc.tile_pool(name="work", bufs=3) as wpool,
            tc.tile_pool(name="gates", bufs=1, space="PSUM") as gpool,
            tc.tile_pool(name="trp", bufs=1, space="PSUM") as trpool,
        ):
            ident = cpool.tile([BLOC, BLOC], bf16, tag="ident")
            make_identity(nc, ident[:])

            # state first (small, needed immediately)
            h = spool.tile([BLOC, D], bf16, tag="h")
            nc.sync.dma_start(h[:], h0_d[:])
            hT = []
            for k in range(KC):
                t_ = stpool.tile([P, BLOC], f32r, tag=f"hT{k}")
                nc.sync.dma_start(t_[:], h0T_d[:, k * BLOC : (k + 1) * BLOC])
                hT.append(t_)

            # r-bank weights arrive per k-chunk (separate tiles, so step 1's
            # first matmuls start after 256KB instead of 1MB); chunk 0 is
            # further split into 4 parallel sub-DMAs (different queues) so
            # the very first matmul starts ~4x earlier.
            vr_sb = []
            for k in range(KC):
                vk = cpool.tile([P, 512], f32r, tag=f"vr{k}")
                if k == 0:
                    for j in range(4):
                        nc.sync.dma_start(
                            vk[:, j * 128 : (j + 1) * 128],
                            vr_d[:, j * 128 : (j + 1) * 128],
                        )
                else:
                    nc.sync.dma_start(vk[:], vr_d[:, k * 512 : (k + 1) * 512])
                vr_sb.append(vk)
            vhh_sb = cpool.tile([P, KC * 512], f32r, tag="vhh")
            vxh_sb = cpool.tile([P, KC * 512], f32r, tag="vxh")
            vz0_sb = cpool.tile([P, KC * 256], f32r, tag="vz0")
            vz1_sb = cpool.tile([P, KC * 256], f32r, tag="vz1")
            nc.sync.dma_start(vhh_sb[:], vhh_d[:])
            nc.sync.dma_start(vxh_sb[:], vxh_d[:])
            nc.sync.dma_start(vz0_sb[:], vz0_d[:])
            nc.sync.dma_start(vz1_sb[:], vz1_d[:])
            if has_bias:
                bias_sb = cpool.tile([BLOC, GW], f32, tag="bias")
                nc.sync.dma_start(bias_sb[:], bias_d[:])

            # PE warm-up on locally-built identity (no DMA dependence):
            # flips the HAM clock gate to K=8/8 while weights stream in.
            # Targets the trp scratch (overwritten by real transposes later).
            wu = trpool.tile([P, KC * BLOC], bf16, tag="trpA", name="wu")
            for i in range(24):
                nc.tensor.matmul(
                    wu[:BLOC, (i % KC) * BLOC : (i % KC + 1) * BLOC],
                    ident[:],
                    ident[:],
                    is_transpose=True,
                    start=True,
                    stop=True,
                )

            for t in range(TDEV):
                last = t == TDEV - 1
                # ---- gate matmul stream ----
                gr = gpool.tile([BLOC, 512], f32, tag="gr", name="gr")
                for k in range(KC):
                    nc.tensor.matmul(
                        gr[:],
                        hT[k][:],
                        vr_sb[k][:],
                        start=(k == 0),
                        stop=(k == KC - 1),
                    )
                if has_bias:
                    nc.vector.tensor_add(gr[:], gr[:], bias_sb[:, 0:512])
                r = wpool.tile([BLOC, D], bf16, tag="r", name="r")
                nc.scalar.activation(r[:], gr[:], AF.Sigmoid)

                ghh = gpool.tile([BLOC, 512], f32, tag="ghh", name="ghh")
                for k in range(KC):
                    nc.tensor.matmul(
                        ghh[:],
                        hT[k][:],
                        vhh_sb[:, k * 512 : (k + 1) * 512],
                        start=(k == 0),
                        stop=(k == KC - 1),
                    )
                if has_bias:
                    nc.vector.tensor_add(ghh[:], ghh[:], bias_sb[:, 512:1024])
                p = wpool.tile([BLOC, D], bf16, tag="p", name="p")
                nc.vector.tensor_mul(p[:], r[:], ghh[:])

                gxh = gpool.tile([BLOC, 512], f32, tag="gxh", name="gxh")
                for k in range(KC):
                    nc.tensor.matmul(
                        gxh[:],
                        hT[k][:],
                        vxh_sb[:, k * 512 : (k + 1) * 512],
                        start=(k == 0),
                        stop=False,
                    )
                # q = xh + p, accumulated by the PE (ident.T @ p == p)
                nc.tensor.matmul(gxh[:], ident[:], p[:], start=False, stop=True)
                if has_bias:
                    nc.vector.tensor_add(gxh[:], gxh[:], bias_sb[:, 1024:1536])
                hhat = wpool.tile([BLOC, D], bf16, tag="hhat", name="hhat")
                nc.scalar.activation(hhat[:], gxh[:], AF.Tanh)

                gz0 = gpool.tile([BLOC, 256], f32, tag="gz0", name="gz0")
                gz1 = gpool.tile([BLOC, 256], f32, tag="gz1", name="gz1")
                for gz, coff in ((gz0, 1536), (gz1, 1792)):
                    for k in range(KC):
                        nc.tensor.matmul(
                            gz[:],
                            hT[k][:],
                            (vz0_sb if coff == 1536 else vz1_sb)[
                                :, k * 256 : (k + 1) * 256
                            ],
                            start=(k == 0),
                            stop=(k == KC - 1),
                        )
                    if has_bias:
                        nc.vector.tensor_add(
                            gz[:], gz[:], bias_sb[:, coff : coff + 256]
                        )

                if not last:
                    # hhat^T chunks (transpose-mode; PE reaches these right
                    # after the z mms — tanh is done thanks to the q-matmul).
                    # NOTE a regular-matmul transpose (lhsT=chunk, rhs=I)
                    # accumulating tt^T on top does NOT work: two open
                    # accumulation groups cannot share a PSUM bank.
                    trpA = trpool.tile([P, KC * BLOC], bf16, tag="trpA", name="trpA")
                    for k in range(KC):
                        nc.tensor.matmul(
                            trpA[:, k * BLOC : (k + 1) * BLOC],
                            hhat[:, k * P : (k + 1) * P],
                            ident[:],
                            is_transpose=True,
                            start=True,
                            stop=True,
                        )

                z0 = wpool.tile([BLOC, 256], bf16, tag="z0", name="z0")
                z1 = wpool.tile([BLOC, 256], bf16, tag="z1", name="z1")
                nc.scalar.activation(z0[:], gz0[:], AF.Sigmoid)
                nc.scalar.activation(z1[:], gz1[:], AF.Sigmoid)

                s0 = wpool.tile([BLOC, 256], bf16, tag="s0", name="s0")
                s1 = wpool.tile([BLOC, 256], bf16, tag="s1", name="s1")
                nc.vector.tensor_sub(s0[:], h[:, 0:256], hhat[:, 0:256])
                nc.vector.tensor_sub(s1[:], h[:, 256:512], hhat[:, 256:512])

                tts = []
                for k in range(KC):
                    tts.append(
                        wpool.tile([BLOC, P], bf16, tag=f"tt{k}", name=f"tt{k}")
                    )

                def make_tt(k):
                    zt = z0 if k < 2 else z1
                    st = s0 if k < 2 else s1
                    off = (k % 2) * P
                    nc.vector.tensor_mul(
                        tts[k][:], zt[:, off : off + P], st[:, off : off + P]
                    )

                h_new = spool.tile([BLOC, D], bf16, tag="h")
                if not last:
                    # tt^T chunks, then hT_k = ScalarE-cast(hhat^T_k) + DVE
                    # += tt^T_k (dual-PSUM TensorTensor is rejected by the
                    # BIR verifier; GpSimd tips the chip into P0 — 20% slower
                    # everywhere). h_new assembly is issued last.
                    trpB = [
                        trpool.tile([P, 2 * BLOC], bf16, tag="trpB01", name="trpB01"),
                        trpool.tile([P, 2 * BLOC], bf16, tag="trpB23", name="trpB23"),
                    ]
                    hT_new = [None] * KC
                    for k in range(KC):
                        make_tt(k)
                        nc.tensor.matmul(
                            trpB[k // 2][:, (k % 2) * BLOC : (k % 2 + 1) * BLOC],
                            tts[k][:],
                            ident[:],
                            is_transpose=True,
                            start=True,
                            stop=True,
                        )
                        if k % 2 == 1:
                            for kk in (k - 1, k):
                                tn = stpool.tile([P, BLOC], f32r, tag=f"hT{kk}")
                                nc.scalar.copy(
                                    tn[:], trpA[:, kk * BLOC : (kk + 1) * BLOC]
                                )
                                nc.vector.tensor_add(
                                    tn[:],
                                    tn[:],
                                    trpB[k // 2][:, (kk % 2) * BLOC : (kk % 2 + 1) * BLOC],
                                )
                                hT_new[kk] = tn
                    hT = hT_new
                else:
                    for k in range(KC):
                        make_tt(k)
                for k in range(KC):
                    nc.vector.tensor_add(
                        h_new[:, k * P : (k + 1) * P],
                        hhat[:, k * P : (k + 1) * P],
                        tts[k][:],
                    )
                nc.sync.dma_start(out_d[:, t, :], h_new[:])
                h = h_new

    nc.compile()
    return nc


def kernel(x, W, U, b):
    from concourse.bass_utils import run_bass_kernel_spmd

    x = np.asarray(x, dtype=np.float32)
    W = np.asarray(W, dtype=np.float32)
    U = np.asarray(U, dtype=np.float32)
    b = np.asarray(b, dtype=np.float32)

    V, _, bias = _prepare_weights(W, U, b)
    has_bias = bool(np.any(bias != 0.0))
    vr = _bank_layout(V[:, 0:512]).astype(np.float32)
    vhh = _bank_layout(V[:, 512:1024]).astype(np.float32)
    vxh = _bank_layout(V[:, 1024:1536]).astype(np.float32)
    vz0 = _bank_layout(V[:, 1536:1792]).astype(np.float32)
    vz1 = _bank_layout(V[:, 1792:2048]).astype(np.float32)

    h1 = _host_step0(x, W, U, b)  # [B, D] f32

    key = ("gru_v1", has_bias)
    if key not in _CACHE:
        _CACHE[key] = _build(has_bias)
    nc = _CACHE[key]

    in_maps = []
    for i in range(NCORES):
        hs = h1[i * BLOC : (i + 1) * BLOC]  # [64, 512] f32
        hs_b = hs.astype(_BF16)
        m = {
            "vr": vr,
            "vhh": vhh,
            "vxh": vxh,
            "vz0": vz0,
            "vz1": vz1,
            "h0": hs_b,
            "h0T": np.ascontiguousarray(
                hs_b.astype(np.float32)
                .reshape(BLOC, KC, P)
                .transpose(2, 1, 0)
                .reshape(P, KC * BLOC)
            ),
        }
        if has_bias:
            m["bias"] = np.ascontiguousarray(
                np.broadcast_to(bias[None, :], (BLOC, GW))
            ).astype(np.float32)
        in_maps.append(m)

    res = run_bass_kernel_spmd(
        nc, in_maps, core_ids=list(range(NCORES)), trace=TRACE, tmpdir=TMPDIR
    )
    LAST["exec_time_ns"] = res.exec_time_ns
    LAST["results"] = res
    dev = np.concatenate(
        [np.asarray(res.results[i]["out"]) for i in range(NCORES)], axis=0
    )  # [B, T-1, D] bf16
    out = np.empty((B, T, D), dtype=np.float32)
    out[:, 0, :] = h1
    out[:, 1:, :] = dev.astype(np.float32)
    return out


# revision 4
# speedup vs baseline: 1.0137x; 1.0137x over previous
"""Autoregressive GRU on 8 TRN2 NeuronCores — V1 pipeline rework.

Data-parallel: batch B=512 split 64 rows/core; T=128 sequential steps local
per core. Step 0 (zero input) is computed on the HOST (no v0 weight matrix,
one less device step); the device runs steps 1..127 with one fused weight.

Key algebra (Keras GRU, reset_after=True, gate order [z, r, h]):
  step t>=1: inp == h  ->  one matmul against host-prefused
  V = [Wr+Ur | Uh | Wh | Wz+Uz]  [D, 4D], bank order [r | hh | xh | z], then
  r = sigmoid(rb); hhat = tanh(xhb + r*hhb); z = sigmoid(zb)
  h_new = hhat + z*(h - hhat)

Measured: 780-785us (vs 906us for the prior baseline on the same machine
state; converged step 5905ns vs 6490ns, preamble 14.3us vs 20.9us, and the
every-4th-step HAM clock re-throttle mostly gone: 3-4 slow steps vs 33).
Rel err 8.1e-3 (gate 2e-2). NOTE the machine has a chip-wide power state
that varies run to run (~20%: matmul p50 390ns vs 466ns) — absolute times
fluctuate accordingly.

Changes vs the 1.08ms baseline (measured step 6490ns):
- q = xh + r*hh is accumulated BY THE PE: a 5th matmul ident.T @ p with
  start=False into the open xh PSUM bank. Drops a 688ns DVE op from the
  chain and starts tanh ~450ns earlier.
- z-bank is split into two 256-wide PSUM tiles (f32r still 1 cycle/row at
  N=256) so sigmoid(z0) starts a half-bank earlier.
- The recurrent state hT is 4 chunk tiles [128,64]; per chunk:
  tt_k = z*s chunk -> PE transpose -> hT_k = copy(hhat^T_k) + tt^T_k
  (copy on ScalarE, add on DVE), so the NEXT step's k-th matmul starts as
  soon as chunk k is assembled instead of after the full [128,256] state.
- Output DMA'd as bf16 straight from h_new (host casts to f32; h is already
  bf16 so values are identical) — frees the 721ns/step ScalarE f32 copy.
- Weights DMA'd per-bank (r-bank per k-chunk, first chunk as 4 parallel
  sub-DMAs) so step 1's matmuls start after 64KB, not 4MB.

Things measured NOT to work (do not retry):
- h_new/tt/s on GpSimd: any extra Pool-engine activity tips the chip into
  the P0 power state — EVERY engine slows ~20% (ACT 682->818ns).
- hT_k = trpA_k + trpB_k as one DVE add: the BIR verifier rejects
  TensorTensor with two PSUM sources (hence the ScalarE cast + DVE add).
- Transposes as REGULAR matmuls (lhsT=chunk, rhs=I) to accumulate
  hhat^T + tt^T in PSUM without the add: two open accumulation groups
  cannot share a PSUM bank -> garbage output.
- sigma(z1) issued after the chunk-0/1 casts on ScalarE: the delayed z1
  path stalls next-step k2/k3 matmuls and the stream slip compounds
  (875us vs 785us).
- PSUM pool tiles are bank-granular (8 banks x 2KB): 5 gate tiles +
  trpA + trpB01 + trpB23 is exactly 8 — no room for finer chunking.
"""

import numpy as np
import ml_dtypes

B, D, T = 512, 512, 128
NCORES = 8
BLOC = B // NCORES  # 64
P = 128
KC = D // P  # 4 K-chunks
GW = 4 * D  # 2048 fused gate columns: [r | hh | xh | z]

_BF16 = ml_dtypes.bfloat16

TRACE = False
TMPDIR = None
LAST = {}


def _prepare_weights(W, U, b):
    """Host-side fusion. Returns (V, V0, bias) in math layout.

    V bank order [r | hh | xh | z]; V0 kept for numpy_sim compatibility."""
    Wz, Wr, Wh = W[:, :D], W[:, D : 2 * D], W[:, 2 * D :]
    Uz, Ur, Uh = U[:, :D], U[:, D : 2 * D], U[:, 2 * D :]
    V = np.concatenate([Wr + Ur, Uh, Wh, Wz + Uz], axis=1)  # [D, GW]
    V0 = np.concatenate([Ur, Uh, np.zeros_like(Wh), Uz], axis=1)
    b0, b1 = b[0], b[1]
    bias = np.concatenate(
        [b0[D : 2 * D] + b1[D : 2 * D], b1[2 * D :], b0[2 * D :], b0[:D] + b1[:D]]
    )  # [GW], order [r | hh | xh | z]
    return V, V0, bias


def _bank_layout(Vb):
    # Vb [D, w] -> dev [P, KC*w]: dev[p, k*w + j] = Vb[k*128 + p, j]
    w = Vb.shape[1]
    return np.ascontiguousarray(
        Vb.reshape(KC, P, w).transpose(1, 0, 2).reshape(P, KC * w)
    )


def _host_step0(x, W, U, b):
    """h1 = gru_cell(inp=0, h=x) in f32 on the host."""
    gx = np.broadcast_to(b[0], (x.shape[0], 3 * D))  # 0 @ W + b0
    gh = x @ U + b[1]
    xz, xr, xh = gx[:, :D], gx[:, D : 2 * D], gx[:, 2 * D :]
    hz, hr, hh = gh[:, :D], gh[:, D : 2 * D], gh[:, 2 * D :]
    z = 1.0 / (1.0 + np.exp(-(xz + hz)))
    r = 1.0 / (1.0 + np.exp(-(xr + hr)))
    hhat = np.tanh(xh + r * hh)
    return z * x + (1.0 - z) * hhat


_CACHE = {}


def _build(has_bias: bool):
    import concourse.mybir as mybir
    import concourse.tile as tile
    from concourse import bacc
    from concourse.masks import make_identity

    f32 = mybir.dt.float32
    f32r = mybir.dt.float32r
    bf16 = mybir.dt.bfloat16
    AF = mybir.ActivationFunctionType

    TDEV = T - 1  # 127 device steps

    nc = bacc.Bacc(
        "TRN2", target_bir_lowering=False, debug=False, num_devices=NCORES
    )
    vr_d = nc.dram_tensor("vr", [P, KC * 512], f32r, kind="ExternalInput").ap()
    vhh_d = nc.dram_tensor("vhh", [P, KC * 512], f32r, kind="ExternalInput").ap()
    vxh_d = nc.dram_tensor("vxh", [P, KC * 512], f32r, kind="ExternalInput").ap()
    vz0_d = nc.dram_tensor("vz0", [P, KC * 256], f32r, kind="ExternalInput").ap()
    vz1_d = nc.dram_tensor("vz1", [P, KC * 256], f32r, kind="ExternalInput").ap()
    h0_d = nc.dram_tensor("h0", [BLOC, D], bf16, kind="ExternalInput").ap()
    h0T_d = nc.dram_tensor("h0T", [P, KC * BLOC], f32r, kind="ExternalInput").ap()
    if has_bias:
        bias_d = nc.dram_tensor("bias", [BLOC, GW], f32, kind="ExternalInput").ap()
    out_d = nc.dram_tensor("out", [BLOC, TDEV, D], bf16, kind="ExternalOutput").ap()

    with tile.TileContext(nc) as tc:
        with (
            tc.tile_pool(name="const", bufs=1) as cpool,
            tc.tile_pool(name="state", bufs=3) as spool,
            tc.tile_pool(name="statet", bufs=2) as stpool,
            tc.tile_pool(name="work", bufs=3) as wpool,
            tc.tile_pool(name="gates", bufs=1, space="PSUM") as gpool,
            tc.tile_pool(name="trp", bufs=1, space="PSUM") as trpool,
        ):
            ident = cpool.tile([BLOC, BLOC], bf16, tag="ident")
            make_identity(nc, ident[:])

            # state first (small, needed immediately)
            h = spool.tile([BLOC, D], bf16, tag="h")
            nc.sync.dma_start(h[:], h0_d[:])
            hT = []
            for k in range(KC):
                t_ = stpool.tile([P, BLOC], f32r, tag=f"hT{k}")
                nc.sync.dma_start(t_[:], h0T_d[:, k * BLOC : (k + 1) * BLOC])
                hT.append(t_)

            # r-bank weights arrive per k-chunk (separate tiles, so step 1's
            # first matmuls start after 256KB instead of 1MB); chunk 0 is
            # further split into 4 parallel sub-DMAs (different queues) so
            # the very first matmul starts ~4x earlier.
            vr_sb = []
            for k in range(KC):
                vk = cpool.tile([P, 512], f32r, tag=f"vr{k}")
                if k == 0:
                    for j in range(4):
                        nc.sync.dma_start(
                            vk[:, j * 128 : (j + 1) * 128],
                            vr_d[:, j * 128 : (j + 1) * 128],
                        )
                else:
                    nc.sync.dma_start(vk[:], vr_d[:, k * 512 : (k + 1) * 512])
                vr_sb.append(vk)
            vhh_sb = cpool.tile([P, KC * 512], f32r, tag="vhh")
            vxh_sb = cpool.tile([P, KC * 512], f32r, tag="vxh")
            vz0_sb = cpool.tile([P, KC * 256], f32r, tag="vz0")
            vz1_sb = cpool.tile([P, KC * 256], f32r, tag="vz1")
            nc.sync.dma_start(vhh_sb[:], vhh_d[:])
            nc.sync.dma_start(vxh_sb[:], vxh_d[:])
            nc.sync.dma_start(vz0_sb[:], vz0_d[:])
            nc.sync.dma_start(vz1_sb[:], vz1_d[:])
            if has_bias:
                bias_sb = cpool.tile([BLOC, GW], f32, tag="bias")
                nc.sync.dma_start(bias_sb[:], bias_d[:])

            # PE warm-up on locally-built identity (no DMA dependence):
            # flips the HAM clock gate to K=8/8 while weights stream in.
            # Targets the trp scratch (overwritten by real transposes later).
            wu = trpool.tile([P, KC * BLOC], bf16, tag="trpA", name="wu")
            for i in range(24):
                nc.tensor.matmul(
                    wu[:BLOC, (i % KC) * BLOC : (i % KC + 1) * BLOC],
                    ident[:],
                    ident[:],
                    is_transpose=True,
                    start=True,
                    stop=True,
                )

            for t in range(TDEV):
                last = t == TDEV - 1
                # ---- gate matmul stream ----
                gr = gpool.tile([BLOC, 512], f32, tag="gr", name="gr")
                for k in range(KC):
                    nc.tensor.matmul(
                        gr[:],
                        hT[k][:],
                        vr_sb[k][:],
                        start=(k == 0),
                        stop=(k == KC - 1),
                    )
                if has_bias:
                    nc.vector.tensor_add(gr[:], gr[:], bias_sb[:, 0:512])
                r = wpool.tile([BLOC, D], bf16, tag="r", name="r")
                nc.scalar.activation(r[:], gr[:], AF.Sigmoid)

                ghh = gpool.tile([BLOC, 512], f32, tag="ghh", name="ghh")
                for k in range(KC):
                    nc.tensor.matmul(
                        ghh[:],
                        hT[k][:],
                        vhh_sb[:, k * 512 : (k + 1) * 512],
                        start=(k == 0),
                        stop=(k == KC - 1),
                    )
                if has_bias:
                    nc.vector.tensor_add(ghh[:], ghh[:], bias_sb[:, 512:1024])
                p = wpool.tile([BLOC, D], bf16, tag="p", name="p")
                nc.vector.tensor_mul(p[:], r[:], ghh[:])

                gxh = gpool.tile([BLOC, 512], f32, tag="gxh", name="gxh")
                for k in range(KC):
                    nc.tensor.matmul(
                        gxh[:],
                        hT[k][:],
                        vxh_sb[:, k * 512 : (k + 1) * 512],
                        start=(k == 0),
                        stop=False,
                    )
                # q = xh + p, accumulated by the PE (ident.T @ p == p)
                nc.tensor.matmul(gxh[:], ident[:], p[:], start=False, stop=True)
                if has_bias:
                    nc.vector.tensor_add(gxh[:], gxh[:], bias_sb[:, 1024:1536])
                hhat = wpool.tile([BLOC, D], bf16, tag="hhat", name="hhat")
                nc.scalar.activation(hhat[:], gxh[:], AF.Tanh)

                gz0 = gpool.tile([BLOC, 256], f32, tag="gz0", name="gz0")
                gz1 = gpool.tile([BLOC, 256], f32, tag="gz1", name="gz1")
                for gz, coff in ((gz0, 1536), (gz1, 1792)):
                    for k in range(KC):
                        nc.tensor.matmul(
                            gz[:],
                            hT[k][:],
                            (vz0_sb if coff == 1536 else vz1_sb)[
                                :, k * 256 : (k + 1) * 256
                            ],
                            start=(k == 0),
                            stop=(k == KC - 1),
                        )
                    if has_bias:
                        nc.vector.tensor_add(
                            gz[:], gz[:], bias_sb[:, coff : coff + 256]
                        )

                if not last:
                    # hhat^T chunks (transpose-mode; PE reaches these right
                    # after the z mms — tanh is done thanks to the q-matmul).
                    # NOTE a regular-matmul transpose (lhsT=chunk, rhs=I)
                    # accumulating tt^T on top does NOT work: two open
                    # accumulation groups cannot share a PSUM bank.
                    trpA = trpool.tile([P, KC * BLOC], bf16, tag="trpA", name="trpA")
                    for k in range(KC):
                        nc.tensor.matmul(
                            trpA[:, k * BLOC : (k + 1) * BLOC],
                            hhat[:, k * P : (k + 1) * P],
                            ident[:],
                            is_transpose=True,
                            start=True,
                            stop=True,
                        )

                z0 = wpool.tile([BLOC, 256], bf16, tag="z0", name="z0")
                z1 = wpool.tile([BLOC, 256], bf16, tag="z1", name="z1")
                nc.scalar.activation(z0[:], gz0[:], AF.Sigmoid)
                nc.scalar.activation(z1[:], gz1[:], AF.Sigmoid)

                s0 = wpool.tile([BLOC, 256], bf16, tag="s0", name="s0")
                s1 = wpool.tile([BLOC, 256], bf16, tag="s1", name="s1")
                nc.vector.tensor_sub(s0[:], h[:, 0:256], hhat[:, 0:256])
                nc.vector.tensor_sub(s1[:], h[:, 256:512], hhat[:, 256:512])

                tts = []
                for k in range(KC):
                    tts.append(
                        wpool.tile([BLOC, P], bf16, tag=f"tt{k}", name=f"tt{k}")
                    )

                def make_tt(k):
                    zt = z0 if k < 2 else z1
                    st = s0 if k < 2 else s1
                    off = (k % 2) * P
                    nc.vector.tensor_mul(
                        tts[k][:], zt[:, off : off + P], st[:, off : off + P]
                    )

                h_new = spool.tile([BLOC, D], bf16, tag="h")
                if not last:
                    # tt^T chunks, then hT_k = ScalarE-cast(hhat^T_k) + DVE
                    # += tt^T_k (dual-PSUM TensorTensor is rejected by the
                    # BIR verifier; GpSimd tips the chip into P0 — 20% slower
                    # everywhere). h_new assembly is issued last.
                    trpB = [
                        trpool.tile([P, 2 * BLOC], bf16, tag="trpB01", name="trpB01"),
                        trpool.tile([P, 2 * BLOC], bf16, tag="trpB23", name="trpB23"),
                    ]
                    hT_new = [None] * KC
                    for k in range(KC):
                        make_tt(k)
                        nc.tensor.matmul(
                            trpB[k // 2][:, (k % 2) * BLOC : (k % 2 + 1) * BLOC],
                            tts[k][:],
                            ident[:],
                            is_transpose=True,
                            start=True,
                            stop=True,
                        )
                        if k % 2 == 1:
                            for kk in (k - 1, k):
                                tn = stpool.tile([P, BLOC], f32r, tag=f"hT{kk}")
                                nc.scalar.copy(
                                    tn[:], trpA[:, kk * BLOC : (kk + 1) * BLOC]
                                )
                                nc.vector.tensor_add(
                                    tn[:],
                                    tn[:],
                                    trpB[k // 2][:, (kk % 2) * BLOC : (kk % 2 + 1) * BLOC],
                                )
                                hT_new[kk] = tn
                    hT = hT_new
                else:
                    for k in range(KC):
                        make_tt(k)
                for k in range(KC):
                    nc.vector.tensor_add(
                        h_new[:, k * P : (k + 1) * P],
                        hhat[:, k * P : (k + 1) * P],
                        tts[k][:],
                    )
                nc.sync.dma_start(out_d[:, t, :], h_new[:])
                h = h_new

    nc.compile()
    return nc


def kernel(x, W, U, b):
    from concourse.bass_utils import run_bass_kernel_spmd

    x = np.asarray(x, dtype=np.float32)
    W = np.asarray(W, dtype=np.float32)
    U = np.asarray(U, dtype=np.float32)
    b = np.asarray(b, dtype=np.float32)

    V, _, bias = _prepare_weights(W, U, b)
    has_bias = bool(np.any(bias != 0.0))
    vr = _bank_layout(V[:, 0:512]).astype(np.float32)
    vhh = _bank_layout(V[:, 512:1024]).astype(np.float32)
    vxh = _bank_layout(V[:, 1024:1536]).astype(np.float32)
    vz0 = _bank_layout(V[:, 1536:1792]).astype(np.float32)
    vz1 = _bank_layout(V[:, 1792:2048]).astype(np.float32)

    h1 = _host_step0(x, W, U, b)  # [B, D] f32

    key = ("gru_v1", has_bias)
    if key not in _CACHE:
        _CACHE[key] = _build(has_bias)
    nc = _CACHE[key]

    in_maps = []
    for i in range(NCORES):
        hs = h1[i * BLOC : (i + 1) * BLOC]  # [64, 512] f32
        hs_b = hs.astype(_BF16)
        m = {
            "vr": vr,
            "vhh": vhh,
            "vxh": vxh,
            "vz0": vz0,
            "vz1": vz1,
            "h0": hs_b,
            "h0T": np.ascontiguousarray(
                hs_b.astype(np.float32)
                .reshape(BLOC, KC, P)
                .transpose(2, 1, 0)
                .reshape(P, KC * BLOC)
            ),
        }
        if has_bias:
            m["bias"] = np.ascontiguousarray(
                np.broadcast_to(bias[None, :], (BLOC, GW))
            ).astype(np.float32)
        in_maps.append(m)

    res = run_bass_kernel_spmd(
        nc, in_maps, core_ids=list(range(NCORES)), trace=TRACE, tmpdir=TMPDIR
    )
    LAST["exec_time_ns"] = res.exec_time_ns
    LAST["results"] = res
    dev = np.concatenate(
        [np.asarray(res.results[i]["out"]) for i in range(NCORES)], axis=0
    )  # [B, T-1, D] bf16
    out = np.empty((B, T, D), dtype=np.float32)
    out[:, 0, :] = h1
    out[:, 1:, :] = dev.astype(np.float32)
    return out


# revision 7
# speedup vs baseline: 1.2055x; 1.1892x over previous
"""Autoregressive GRU on 8 TRN2 NeuronCores — V1 pipeline rework.

Data-parallel: batch B=512 split 64 rows/core; T=128 sequential steps local
per core. Step 0 (zero input) is computed on the HOST (no v0 weight matrix,
one less device step); the device runs steps 1..127 with one fused weight.

Key algebra (Keras GRU, reset_after=True, gate order [z, r, h]):
  step t>=1: inp == h  ->  one matmul against host-prefused
  V = [Wr+Ur | Uh | Wh | Wz+Uz]  [D, 4D], bank order [r | hh | xh | z], then
  r = sigmoid(rb); hhat = tanh(xhb + r*hhb); z = sigmoid(zb)
  h_new = hhat + z*(h - hhat)

Measured: 780-785us at full chip clock (vs 906us prior baseline; converged
step 5850-5910ns vs 6490ns, preamble ~14.5us vs 20.9us, every-4th-step HAM
clock re-throttle mostly gone). Rel err 8.1e-3 (gate 2e-2). NOTE the machine
flips between chip-wide power states run-to-run (~20%: gate-matmul p50
390ns vs 466ns) — compare runs via period_p50 * (389 / mm_dur_p50), which
reproduces to ~1% across states for identical code.

Changes vs the 1.08ms baseline (measured step 6490ns):
- tt = z*(h-hhat) and h_new = hhat+tt computed as HALF-tiles (2 DVE ops
  each, not 4): the tail window between sigma(z0) and the next step's first
  matmul is DVE-saturated (the list scheduler backfills every ready op
  ahead of the chain-critical hT adds), so fewer/larger ops shrink it.
  Normalized step 5850 vs 5905 with quarters.
- q = xh + r*hh is accumulated BY THE PE: a 5th matmul ident.T @ p with
  start=False into the open xh PSUM bank. Drops a 688ns DVE op from the
  chain and starts tanh ~450ns earlier.
- z-bank is split into two 256-wide PSUM tiles (f32r still 1 cycle/row at
  N=256) so sigmoid(z0) starts a half-bank earlier.
- The recurrent state hT is 4 chunk tiles [128,64]; per chunk:
  tt_k = z*s chunk -> PE transpose -> hT_k = copy(hhat^T_k) + tt^T_k
  (copy on ScalarE, add on DVE), so the NEXT step's k-th matmul starts as
  soon as chunk k is assembled instead of after the full [128,256] state.
- Output DMA'd as bf16 straight from h_new (host casts to f32; h is already
  bf16 so values are identical) — frees the 721ns/step ScalarE f32 copy.
- Weights DMA'd per-bank (r-bank per k-chunk, first chunk as 4 parallel
  sub-DMAs) so step 1's matmuls start after 64KB, not 4MB.

Things measured NOT to work (do not retry):
- h_new/tt/s on GpSimd: ANY extra Pool-engine activity tips the chip into
  the P0 power state — EVERY engine slows ~20% (ACT 682->818ns).
- hT_k = trpA_k + trpB_k as one DVE add: the BIR verifier rejects
  TensorTensor with two PSUM sources (hence ScalarE cast + DVE add).
- Transposes as REGULAR matmuls (lhsT=chunk, rhs=I) to accumulate
  hhat^T + tt^T in PSUM without the add: two open accumulation groups
  cannot share a PSUM bank -> garbage output.
- sigma(z1) issued after the chunk-0/1 casts on ScalarE: stalls next-step
  k2/k3 matmuls; the stream slip compounds (norm. step 6638 vs 5905).
- PSUM pool tiles are bank-granular (8 banks x 2KB): 5 gate tiles + trpA +
  trpB01 + trpB23 is exactly 8 — no room for finer splits (xh halves for
  an earlier tanh would need a 9th bank).
- Cross-engine dependency hops cost ~150-250ns wall (sem update fires
  after pipeline drain), not the nominal ~50ns — op-count on the
  ACT->DVE->PE->DVE tail chain matters more than op size.
"""

import numpy as np
import ml_dtypes

B, D, T = 512, 512, 128
NCORES = 8
BLOC = B // NCORES  # 64
P = 128
KC = D // P  # 4 K-chunks
GW = 4 * D  # 2048 fused gate columns: [r | hh | xh | z]

_BF16 = ml_dtypes.bfloat16

TRACE = False
TMPDIR = None
LAST = {}


def _prepare_weights(W, U, b):
    """Host-side fusion. Returns (V, V0, bias) in math layout.

    V bank order [r | hh | xh | z]; V0 kept for numpy_sim compatibility."""
    Wz, Wr, Wh = W[:, :D], W[:, D : 2 * D], W[:, 2 * D :]
    Uz, Ur, Uh = U[:, :D], U[:, D : 2 * D], U[:, 2 * D :]
    V = np.concatenate([Wr + Ur, Uh, Wh, Wz + Uz], axis=1)  # [D, GW]
    V0 = np.concatenate([Ur, Uh, np.zeros_like(Wh), Uz], axis=1)
    b0, b1 = b[0], b[1]
    bias = np.concatenate(
        [b0[D : 2 * D] + b1[D : 2 * D], b1[2 * D :], b0[2 * D :], b0[:D] + b1[:D]]
    )  # [GW], order [r | hh | xh | z]
    return V, V0, bias


def _bank_layout(Vb):
    # Vb [D, w] -> dev [P, KC*w]: dev[p, k*w + j] = Vb[k*128 + p, j]
    w = Vb.shape[1]
    return np.ascontiguousarray(
        Vb.reshape(KC, P, w).transpose(1, 0, 2).reshape(P, KC * w)
    )


def _host_step0(x, W, U, b):
    """h1 = gru_cell(inp=0, h=x) in f32 on the host."""
    gx = np.broadcast_to(b[0], (x.shape[0], 3 * D))  # 0 @ W + b0
    gh = x @ U + b[1]
    xz, xr, xh = gx[:, :D], gx[:, D : 2 * D], gx[:, 2 * D :]
    hz, hr, hh = gh[:, :D], gh[:, D : 2 * D], gh[:, 2 * D :]
    z = 1.0 / (1.0 + np.exp(-(xz + hz)))
    r = 1.0 / (1.0 + np.exp(-(xr + hr)))
    hhat = np.tanh(xh + r * hh)
    return z * x + (1.0 - z) * hhat


_CACHE = {}


def _build(has_bias: bool):
    import concourse.mybir as mybir
    import concourse.tile as tile
    from concourse import bacc
    from concourse.masks import make_identity

    f32 = mybir.dt.float32
    f32r = mybir.dt.float32r
    bf16 = mybir.dt.bfloat16
    AF = mybir.ActivationFunctionType

    TDEV = T - 1  # 127 device steps

    nc = bacc.Bacc(
        "TRN2", target_bir_lowering=False, debug=False, num_devices=NCORES
    )
    vr_d = nc.dram_tensor("vr", [P, KC * 512], f32r, kind="ExternalInput").ap()
    vhh_d = nc.dram_tensor("vhh", [P, KC * 512], f32r, kind="ExternalInput").ap()
    vxh_d = nc.dram_tensor("vxh", [P, KC * 512], f32r, kind="ExternalInput").ap()
    vz0_d = nc.dram_tensor("vz0", [P, KC * 256], f32r, kind="ExternalInput").ap()
    vz1_d = nc.dram_tensor("vz1", [P, KC * 256], f32r, kind="ExternalInput").ap()
    h0_d = nc.dram_tensor("h0", [BLOC, D], bf16, kind="ExternalInput").ap()
    h0T_d = nc.dram_tensor("h0T", [P, KC * BLOC], f32r, kind="ExternalInput").ap()
    if has_bias:
        bias_d = nc.dram_tensor("bias", [BLOC, GW], f32, kind="ExternalInput").ap()
    out_d = nc.dram_tensor("out", [BLOC, TDEV, D], bf16, kind="ExternalOutput").ap()

    with tile.TileContext(nc) as tc:
        with (
            tc.tile_pool(name="const", bufs=1) as cpool,
            tc.tile_pool(name="state", bufs=3) as spool,
            tc.tile_pool(name="statet", bufs=2) as stpool,
            tc.tile_pool(name="work", bufs=3) as wpool,
            tc.tile_pool(name="gates", bufs=1, space="PSUM") as gpool,
            tc.tile_pool(name="trp", bufs=1, space="PSUM") as trpool,
        ):
            ident = cpool.tile([BLOC, BLOC], bf16, tag="ident")
            make_identity(nc, ident[:])

            # state first (small, needed immediately)
            h = spool.tile([BLOC, D], bf16, tag="h")
            nc.sync.dma_start(h[:], h0_d[:])
            hT = []
            for k in range(KC):
                t_ = stpool.tile([P, BLOC], f32r, tag=f"hT{k}")
                nc.sync.dma_start(t_[:], h0T_d[:, k * BLOC : (k + 1) * BLOC])
                hT.append(t_)

            # r-bank weights arrive per k-chunk (separate tiles, so step 1's
            # first matmuls start after 256KB instead of 1MB); chunk 0 is
            # further split into 4 parallel sub-DMAs (different queues) so
            # the very first matmul starts ~4x earlier.
            vr_sb = []
            for k in range(KC):
                vk = cpool.tile([P, 512], f32r, tag=f"vr{k}")
                if k == 0:
                    for j in range(4):
                        nc.sync.dma_start(
                            vk[:, j * 128 : (j + 1) * 128],
                            vr_d[:, j * 128 : (j + 1) * 128],
                        )
                else:
                    nc.sync.dma_start(vk[:], vr_d[:, k * 512 : (k + 1) * 512])
                vr_sb.append(vk)
            vhh_sb = cpool.tile([P, KC * 512], f32r, tag="vhh")
            vxh_sb = cpool.tile([P, KC * 512], f32r, tag="vxh")
            vz0_sb = cpool.tile([P, KC * 256], f32r, tag="vz0")
            vz1_sb = cpool.tile([P, KC * 256], f32r, tag="vz1")
            nc.sync.dma_start(vhh_sb[:], vhh_d[:])
            nc.sync.dma_start(vxh_sb[:], vxh_d[:])
            nc.sync.dma_start(vz0_sb[:], vz0_d[:])
            nc.sync.dma_start(vz1_sb[:], vz1_d[:])
            if has_bias:
                bias_sb = cpool.tile([BLOC, GW], f32, tag="bias")
                nc.sync.dma_start(bias_sb[:], bias_d[:])

            # PE warm-up on locally-built identity (no DMA dependence):
            # flips the HAM clock gate to K=8/8 while weights stream in.
            # Targets the trp scratch (overwritten by real transposes later).
            wu = trpool.tile([P, KC * BLOC], bf16, tag="trpA", name="wu")
            for i in range(24):
                nc.tensor.matmul(
                    wu[:BLOC, (i % KC) * BLOC : (i % KC + 1) * BLOC],
                    ident[:],
                    ident[:],
                    is_transpose=True,
                    start=True,
                    stop=True,
                )

            for t in range(TDEV):
                last = t == TDEV - 1
                # ---- gate matmul stream ----
                gr = gpool.tile([BLOC, 512], f32, tag="gr", name="gr")
                for k in range(KC):
                    nc.tensor.matmul(
                        gr[:],
                        hT[k][:],
                        vr_sb[k][:],
                        start=(k == 0),
                        stop=(k == KC - 1),
                    )
                if has_bias:
                    nc.vector.tensor_add(gr[:], gr[:], bias_sb[:, 0:512])
                r = wpool.tile([BLOC, D], bf16, tag="r", name="r")
                nc.scalar.activation(r[:], gr[:], AF.Sigmoid)

                ghh = gpool.tile([BLOC, 512], f32, tag="ghh", name="ghh")
                for k in range(KC):
                    nc.tensor.matmul(
                        ghh[:],
                        hT[k][:],
                        vhh_sb[:, k * 512 : (k + 1) * 512],
                        start=(k == 0),
                        stop=(k == KC - 1),
                    )
                if has_bias:
                    nc.vector.tensor_add(ghh[:], ghh[:], bias_sb[:, 512:1024])
                p = wpool.tile([BLOC, D], bf16, tag="p", name="p")
                nc.vector.tensor_mul(p[:], r[:], ghh[:])

                gxh = gpool.tile([BLOC, 512], f32, tag="gxh", name="gxh")
                for k in range(KC):
                    nc.tensor.matmul(
                        gxh[:],
                        hT[k][:],
                        vxh_sb[:, k * 512 : (k + 1) * 512],
                        start=(k == 0),
                        stop=False,
                    )
                # q = xh + p, accumulated by the PE (ident.T @ p == p)
                nc.tensor.matmul(gxh[:], ident[:], p[:], start=False, stop=True)
                if has_bias:
                    nc.vector.tensor_add(gxh[:], gxh[:], bias_sb[:, 1024:1536])
                hhat = wpool.tile([BLOC, D], bf16, tag="hhat", name="hhat")
                nc.scalar.activation(hhat[:], gxh[:], AF.Tanh)

                gz0 = gpool.tile([BLOC, 256], f32, tag="gz0", name="gz0")
                gz1 = gpool.tile([BLOC, 256], f32, tag="gz1", name="gz1")
                for gz, coff in ((gz0, 1536), (gz1, 1792)):
                    for k in range(KC):
                        nc.tensor.matmul(
                            gz[:],
                            hT[k][:],
                            (vz0_sb if coff == 1536 else vz1_sb)[
                                :, k * 256 : (k + 1) * 256
                            ],
                            start=(k == 0),
                            stop=(k == KC - 1),
                        )
                    if has_bias:
                        nc.vector.tensor_add(
                            gz[:], gz[:], bias_sb[:, coff : coff + 256]
                        )

                if not last:
                    # hhat^T chunks (transpose-mode; PE reaches these right
                    # after the z mms — tanh is done thanks to the q-matmul).
                    # NOTE a regular-matmul transpose (lhsT=chunk, rhs=I)
                    # accumulating tt^T on top does NOT work: two open
                    # accumulation groups cannot share a PSUM bank.
                    trpA = trpool.tile([P, KC * BLOC], bf16, tag="trpA", name="trpA")
                    for k in range(KC):
                        nc.tensor.matmul(
                            trpA[:, k * BLOC : (k + 1) * BLOC],
                            hhat[:, k * P : (k + 1) * P],
                            ident[:],
                            is_transpose=True,
                            start=True,
                            stop=True,
                        )

                z0 = wpool.tile([BLOC, 256], bf16, tag="z0", name="z0")
                z1 = wpool.tile([BLOC, 256], bf16, tag="z1", name="z1")
                nc.scalar.activation(z0[:], gz0[:], AF.Sigmoid)
                nc.scalar.activation(z1[:], gz1[:], AF.Sigmoid)

                s0 = wpool.tile([BLOC, 256], bf16, tag="s0", name="s0")
                s1 = wpool.tile([BLOC, 256], bf16, tag="s1", name="s1")
                nc.vector.tensor_sub(s0[:], h[:, 0:256], hhat[:, 0:256])
                nc.vector.tensor_sub(s1[:], h[:, 256:512], hhat[:, 256:512])

                # tt as half-tiles (one DVE op per half, not per quarter:
                # fewer ops in the DVE-saturated tail window)
                ttp = [
                    wpool.tile([BLOC, 256], bf16, tag="tt01", name="tt01"),
                    wpool.tile([BLOC, 256], bf16, tag="tt23", name="tt23"),
                ]

                def make_tt(k):
                    if k % 2 == 0:
                        zt, st = (z0, s0) if k < 2 else (z1, s1)
                        nc.vector.tensor_mul(ttp[k // 2][:], zt[:], st[:])

                h_new = spool.tile([BLOC, D], bf16, tag="h")
                if not last:
                    # tt^T chunks, then hT_k = ScalarE-cast(hhat^T_k) + DVE
                    # += tt^T_k (dual-PSUM TensorTensor is rejected by the
                    # BIR verifier; GpSimd tips the chip into P0 — 20% slower
                    # everywhere). h_new assembly is issued last.
                    trpB = [
                        trpool.tile([P, 2 * BLOC], bf16, tag="trpB01", name="trpB01"),
                        trpool.tile([P, 2 * BLOC], bf16, tag="trpB23", name="trpB23"),
                    ]
                    hT_new = [None] * KC
                    for k in range(KC):
                        make_tt(k)
                        nc.tensor.matmul(
                            trpB[k // 2][:, (k % 2) * BLOC : (k % 2 + 1) * BLOC],
                            ttp[k // 2][:, (k % 2) * P : (k % 2 + 1) * P],
                            ident[:],
                            is_transpose=True,
                            start=True,
                            stop=True,
                        )
                        if k % 2 == 1:
                            for kk in (k - 1, k):
                                tn = stpool.tile([P, BLOC], f32r, tag=f"hT{kk}")
                                nc.scalar.copy(
                                    tn[:], trpA[:, kk * BLOC : (kk + 1) * BLOC]
                                )
                                nc.vector.tensor_add(
                                    tn[:],
                                    tn[:],
                                    trpB[k // 2][:, (kk % 2) * BLOC : (kk % 2 + 1) * BLOC],
                                )
                                hT_new[kk] = tn
                    hT = hT_new
                else:
                    for k in range(KC):
                        make_tt(k)
                for half in range(2):
                    nc.vector.tensor_add(
                        h_new[:, half * 256 : (half + 1) * 256],
                        hhat[:, half * 256 : (half + 1) * 256],
                        ttp[half][:],
                    )
                nc.sync.dma_start(out_d[:, t, :], h_new[:])
                h = h_new

    nc.compile()
    return nc


def kernel(x, W, U, b):
    from concourse.bass_utils import run_bass_kernel_spmd

    x = np.asarray(x, dtype=np.float32)
    W = np.asarray(W, dtype=np.float32)
    U = np.asarray(U, dtype=np.float32)
    b = np.asarray(b, dtype=np.float32)

    V, _, bias = _prepare_weights(W, U, b)
    has_bias = bool(np.any(bias != 0.0))
    vr = _bank_layout(V[:, 0:512]).astype(np.float32)
    vhh = _bank_layout(V[:, 512:1024]).astype(np.float32)
    vxh = _bank_layout(V[:, 1024:1536]).astype(np.float32)
    vz0 = _bank_layout(V[:, 1536:1792]).astype(np.float32)
    vz1 = _bank_layout(V[:, 1792:2048]).astype(np.float32)

    h1 = _host_step0(x, W, U, b)  # [B, D] f32

    key = ("gru_v1", has_bias)
    if key not in _CACHE:
        _CACHE[key] = _build(has_bias)
    nc = _CACHE[key]

    in_maps = []
    for i in range(NCORES):
        hs = h1[i * BLOC : (i + 1) * BLOC]  # [64, 512] f32
        hs_b = hs.astype(_BF16)
        m = {
            "vr": vr,
            "vhh": vhh,
            "vxh": vxh,
            "vz0": vz0,
            "vz1": vz1,
            "h0": hs_b,
            "h0T": np.ascontiguousarray(
                hs_b.astype(np.float32)
                .reshape(BLOC, KC, P)
                .transpose(2, 1, 0)
                .reshape(P, KC * BLOC)
            ),
        }
        if has_bias:
            m["bias"] = np.ascontiguousarray(
                np.broadcast_to(bias[None, :], (BLOC, GW))
            ).astype(np.float32)
        in_maps.append(m)

    res = run_bass_kernel_spmd(
        nc, in_maps, core_ids=list(range(NCORES)), trace=TRACE, tmpdir=TMPDIR
    )
    LAST["exec_time_ns"] = res.exec_time_ns
    LAST["results"] = res
    dev = np.concatenate(
        [np.asarray(res.results[i]["out"]) for i in range(NCORES)], axis=0
    )  # [B, T-1, D] bf16
    out = np.empty((B, T, D), dtype=np.float32)
    out[:, 0, :] = h1
    out[:, 1:, :] = dev.astype(np.float32)
    return out
